# revision 19
# baseline (speedup 1.0000x reference)
"""Trainium2 Bass kernel for nn_AOEncoder (topk_masking).

Strategy (data-parallel over the 544 unified BiLSTM sequences):
  - The model's compute is dominated by two BiLSTMs sharing weights:
    question (B*NQ = 32 seqs) + review (B*K*NR = 512 seqs), all length 64,
    input = [token_emb(128) | aspect_emb(128)], hidden 256, bidirectional.
  - One unified batch of 544 sequences, sharded 68 per NeuronCore
    (4 question + 64 review). Forward and backward chains are staggered so
    PE / ACT / DVE / GPSIMD overlap across the two recurrences.
  - Per step+dir: gates(68,1024) = [e_t; a; hT] x W via K-tile f32r matmuls
    (1 cyc/row), sigmoid/tanh on ACT, c update on DVE+GPSIMD. The next-step
    stationary hT is produced in transposed space: PE transposes c and the
    o-gate, tanh(cT) runs on ACT from PSUM, and h^T = tanh(cT)*oT on DVE.
    h is written to DRAM transposed; the host only untransposes the
    sequences that appear in the output (question + top-k reviews).
  - The self-attention in the reference is degenerate (softmax over a
    size-1 axis => alpha = mask), so attention = masked sum over L; that,
    the bilinear scores, top-k and gathers are tiny and run on host numpy.
"""
import os
import sys
import numpy as np

for _p in ("/opt/trn_rl_repo", "/root/.axon_site/_ro/trn_rl_repo"):
    if _p not in sys.path and os.path.isdir(_p):
        sys.path.append(_p)

B, K, NQ, NR, LQ, LR = 8, 16, 4, 4, 64, 64
D, HID, V, TOPK = 128, 256, 50000, 5
H2 = 2 * HID
G4 = 4 * HID          # 1024 gate channels
SQ = B * NQ           # 32
SR = B * K * NR       # 512
S = SQ + SR           # 544
NCORES = 8
SC = S // NCORES      # 68 sequences per core
QC = SQ // NCORES     # 4 question seqs per core
RC = SR // NCORES     # 64 review seqs per core
L = LQ                # 64 steps

_cache = {}


def _build_nc(with_bias):
    import concourse.bass as bass
    import concourse.tile as tile
    from concourse import bacc, mybir

    f32 = mybir.dt.float32
    f32r = mybir.dt.float32r
    AF = mybir.ActivationFunctionType

    nc = bacc.Bacc(None, target_bir_lowering=False)

    eT_d = nc.dram_tensor("eT", [D, L, SC], f32r, kind="ExternalInput")
    aT_d = nc.dram_tensor("aT", [D, SC], f32r, kind="ExternalInput")
    # W[dir]: (128, 4, 1024): K-tiles [e, a, hT0, hT1] x gate cols [i,f,o,g]
    Wf_d = nc.dram_tensor("Wf", [128, 4, G4], f32r, kind="ExternalInput")
    Wb_d = nc.dram_tensor("Wb", [128, 4, G4], f32r, kind="ExternalInput")
    bias_d = nc.dram_tensor("bias", [1, 2, G4], f32r, kind="ExternalInput")
    ones_d = nc.dram_tensor("ones", [1, SC], f32r, kind="ExternalInput")
    ident_d = nc.dram_tensor("ident", [128, 128], f32r, kind="ExternalInput")

    # h output, transposed: hT[t, j, p, s] = h[s, t, 128*j + p] (per dir)
    hf_d = nc.dram_tensor("hTf", [L, 2, 128, SC], f32r, kind="ExternalOutput")
    hb_d = nc.dram_tensor("hTb", [L, 2, 128, SC], f32r, kind="ExternalOutput")
    # final states: hN transposed (dir, j, p, s); cN normal (dir, s, hid)
    hn_d = nc.dram_tensor("hNT", [2, 2, 128, SC], f32r, kind="ExternalOutput")
    cn_d = nc.dram_tensor("cN", [2, SC, HID], f32r, kind="ExternalOutput")

    with tile.TileContext(nc) as tc:
        with (
            tc.tile_pool(name="const", bufs=1) as constp,
            tc.tile_pool(name="state", bufs=2) as statep,
            tc.tile_pool(name="work", bufs=3) as workp,
            tc.tile_pool(name="psg0", bufs=2, space="PSUM") as psg0,
            tc.tile_pool(name="psg1", bufs=3, space="PSUM") as psg1,
            tc.tile_pool(name="pst", bufs=3, space="PSUM") as pst,
        ):
            eT = constp.tile([D, L, SC], f32r, tag="eT")
            aT = constp.tile([D, SC], f32r, tag="aT")
            W = {
                'f': constp.tile([128, 4, G4], f32r, tag="Wf", name="W_f"),
                'b': constp.tile([128, 4, G4], f32r, tag="Wb", name="W_b"),
            }
            bias_t = constp.tile([1, 2, G4], f32r, tag="bias")
            ones_t = constp.tile([1, SC], f32r, tag="ones")
            ident = constp.tile([128, 128], f32r, tag="ident")
            # first-needed-first: e/a weight K-tiles + boundary eT slices,
            # then identity/bias, then h weight K-tiles, then bulk eT
            nc.sync.dma_start(eT[:, 0, :], eT_d[:, 0, :])
            nc.scalar.dma_start(aT[:], aT_d[:])
            nc.sync.dma_start(W['f'][:, 0, 0:512], Wf_d[:, 0, 0:512])
            nc.scalar.dma_start(W['f'][:, 1, 0:512], Wf_d[:, 1, 0:512])
            nc.gpsimd.dma_start(W['f'][:, 0:2, 512:], Wf_d[:, 0:2, 512:])
            nc.gpsimd.dma_start(eT[:, L - 1, :], eT_d[:, L - 1, :])
            nc.scalar.dma_start(W['b'][:, 0:2, 0:512], Wb_d[:, 0:2, 0:512])
            nc.sync.dma_start(W['b'][:, 0:2, 512:], Wb_d[:, 0:2, 512:])
            nc.sync.dma_start(bias_t[:], bias_d[:])
            nc.sync.dma_start(ones_t[:], ones_d[:])
            nc.sync.dma_start(ident[:], ident_d[:])
            nc.sync.dma_start(W['f'][:, 2:4, :], Wf_d[:, 2:4, :])
            nc.sync.dma_start(W['b'][:, 2:4, :], Wb_d[:, 2:4, :])
            nc.sync.dma_start(eT[:, 1:8, :], eT_d[:, 1:8, :])
            nc.sync.dma_start(eT[:, 56:L - 1, :], eT_d[:, 56:L - 1, :])
            nc.sync.dma_start(eT[:, 8:32, :], eT_d[:, 8:32, :])
            nc.sync.dma_start(eT[:, 32:56, :], eT_d[:, 32:56, :])

            hT = {}
            c = {}
            g_ps = {}

            def mm(t, dr, di):
                te = t if dr == 'f' else (L - 1 - t)
                g0 = psg0.tile([SC, 512], f32, tag="g0", name="g0_" + dr)
                g1 = psg1.tile([SC, 512], f32, tag="g1", name="g1_" + dr)
                g_ps[dr] = (g0, g1)
                for nj, g in enumerate((g0, g1)):
                    nsl = slice(nj * 512, (nj + 1) * 512)
                    nc.tensor.matmul(g[:], eT[:, te, :], W[dr][:, 0, nsl],
                                     start=True, stop=False)
                    last = not (with_bias or t > 0)
                    nc.tensor.matmul(g[:], aT[:], W[dr][:, 1, nsl],
                                     start=False, stop=last)
                    if with_bias:
                        nc.tensor.matmul(g[:], ones_t[:],
                                         bias_t[:, di, nsl],
                                         start=False, stop=not t > 0)
                    if t > 0:
                        nc.tensor.matmul(g[:], hT[dr][:, 0, :],
                                         W[dr][:, 2, nsl],
                                         start=False, stop=False)
                        nc.tensor.matmul(g[:], hT[dr][:, 1, :],
                                         W[dr][:, 3, nsl],
                                         start=False, stop=True)

            def tail(t, dr, di):
                g0, g1 = g_ps[dr]
                # chunk0 = [f, g]; chunk1 = [i, o]
                sf = workp.tile([SC, HID], f32r, tag="sf", name="sf_" + dr)
                nc.scalar.activation(sf[:], g0[:, 0:HID], AF.Sigmoid)
                gg = workp.tile([SC, HID], f32r, tag="gg", name="gg_" + dr)
                nc.scalar.activation(gg[:], g0[:, HID:], AF.Tanh)
                sio = workp.tile([SC, 2 * HID], f32r, tag="sio",
                                 name="sio_" + dr)
                nc.scalar.activation(sio[:, 0:HID], g1[:, 0:HID], AF.Sigmoid)
                nc.scalar.activation(sio[:, HID:], g1[:, HID:], AF.Sigmoid)

                c_new = statep.tile([SC, HID], f32r, tag="c" + dr,
                                    name="c_" + dr)
                if t > 0:
                    fc = workp.tile([SC, HID], f32r, tag="fc",
                                    name="fc_" + dr)
                    nc.gpsimd.tensor_mul(fc[:], sf[:], c[dr][:])
                    ig = workp.tile([SC, HID], f32r, tag="ig",
                                    name="ig_" + dr)
                    nc.vector.tensor_mul(ig[:], sio[:, 0:HID], gg[:])
                    nc.vector.tensor_add(c_new[:], fc[:], ig[:])
                else:
                    nc.vector.tensor_mul(c_new[:], sio[:, 0:HID], gg[:])
                c[dr] = c_new

                # transposed tail: hT = tanh(cT) * oT  (oT, cT share a bank)
                ocT = pst.tile([128, 4, SC], f32r, tag="tps", name="ocT_" + dr)
                nc.tensor.transpose(ocT[:, 0, :], sio[:, HID:HID + 128],
                                    ident[0:SC, 0:SC])
                nc.tensor.transpose(ocT[:, 1, :], sio[:, HID + 128:],
                                    ident[0:SC, 0:SC])
                nc.tensor.transpose(ocT[:, 2, :], c_new[:, 0:128],
                                    ident[0:SC, 0:SC])
                nc.tensor.transpose(ocT[:, 3, :], c_new[:, 128:],
                                    ident[0:SC, 0:SC])
                tcT = workp.tile([128, 2, SC], f32r, tag="tcT",
                                 name="tcT_" + dr)
                nc.scalar.activation(tcT[:], ocT[:, 2:4, :], AF.Tanh)
                hT_new = statep.tile([128, 2, SC], f32r, tag="hT" + dr,
                                     name="hT_" + dr)
                nc.vector.tensor_mul(hT_new[:, 0, :], tcT[:, 0, :],
                                     ocT[:, 0, :])
                nc.vector.tensor_mul(hT_new[:, 1, :], tcT[:, 1, :],
                                     ocT[:, 1, :])
                hT[dr] = hT_new

                # write h (transposed) to DRAM; bwd un-reversed
                h_d = hf_d if dr == 'f' else hb_d
                to = t if dr == 'f' else (L - 1 - t)
                nc.sync.dma_start(
                    h_d[to].rearrange("j p s -> p j s"), hT_new[:])
                if t == L - 1:
                    nc.sync.dma_start(
                        hn_d[di].rearrange("j p s -> p j s"), hT_new[:])
                    nc.sync.dma_start(cn_d[di], c_new[:])

            # software-pipelined emission: fwd and bwd staggered a half step
            mm(0, 'f', 0)
            mm(0, 'b', 1)
            tail(0, 'f', 0)
            for t in range(1, L):
                mm(t, 'f', 0)
                tail(t - 1, 'b', 1)
                mm(t, 'b', 1)
                tail(t, 'f', 0)
            tail(L - 1, 'b', 1)

    nc.compile()
    return nc


def _core_seq_ids(core):
    qs = np.arange(QC * core, QC * (core + 1))
    rs = SQ + np.arange(RC * core, RC * (core + 1))
    return np.concatenate([qs, rs])


def _lstm_trainium(e_seq, asp, params, trace=False):
    """e_seq: (S, L, D) f32, asp: (S, D) f32.
    Returns per-core transposed results:
      hTf, hTb: lists of (L, 2, 128, SC)  [t, j, p, s]
      hNT: list of (2, 2, 128, SC), cN: list of (2, SC, HID)."""
    from concourse.bass_utils import run_bass_kernel_spmd

    (Wih_f, Whh_f, bih_f, bhh_f, Wih_b, Whh_b, bih_b, bhh_b) = params
    f32 = np.float32
    bias_f = (bih_f + bhh_f).astype(f32)
    bias_b = (bih_b + bhh_b).astype(f32)
    with_bias = bool(np.any(bias_f) or np.any(bias_b))

    # gate permutation [f, g, i, o] from pytorch [i, f, g, o]
    perm = np.concatenate([np.arange(HID, 3 * HID),
                           np.arange(0, HID),
                           np.arange(3 * HID, 4 * HID)])

    def pack_W(Wih, Whh):
        # (128, 4, 1024): K-tiles [e, a, hT0, hT1], cols gate-permuted
        Wp_ih = Wih[perm]           # (1024, 256)
        Wp_hh = Whh[perm]           # (1024, 256)
        out = np.empty((128, 4, G4), f32)
        out[:, 0] = Wp_ih[:, 0:D].T
        out[:, 1] = Wp_ih[:, D:2 * D].T
        out[:, 2] = Wp_hh[:, 0:128].T
        out[:, 3] = Wp_hh[:, 128:256].T
        return out

    Wf = pack_W(Wih_f, Whh_f)
    Wb = pack_W(Wih_b, Whh_b)
    bias = np.stack([bias_f[perm], bias_b[perm]])[None]  # (1, 2, 1024)
    ident = np.eye(128, dtype=f32)
    ones = np.ones((1, SC), f32)

    key = with_bias
    if key not in _cache:
        _cache[key] = _build_nc(with_bias)
    nc = _cache[key]

    in_maps = []
    for core in range(NCORES):
        ids = _core_seq_ids(core)
        eT = np.ascontiguousarray(e_seq[ids].transpose(2, 1, 0))  # (D, L, SC)
        aT = np.ascontiguousarray(asp[ids].T)                     # (D, SC)
        in_maps.append({
            "eT": eT, "aT": aT, "Wf": Wf, "Wb": Wb,
            "bias": bias, "ones": ones, "ident": ident,
        })

    res = run_bass_kernel_spmd(nc, in_maps, core_ids=list(range(NCORES)),
                               trace=trace)
    if trace:
        _lstm_trainium.last_exec_ns = res.exec_time_ns

    hTf = [res.results[i]["hTf"] for i in range(NCORES)]
    hTb = [res.results[i]["hTb"] for i in range(NCORES)]
    hNT = [res.results[i]["hNT"] for i in range(NCORES)]
    cN = [res.results[i]["cN"] for i in range(NCORES)]
    return hTf, hTb, hNT, cN


_lstm_trainium.last_exec_ns = None


def _gather_h(hTf, hTb, seq_ids):
    """Assemble h (n, L, H2) for the given global sequence ids from the
    per-core transposed outputs."""
    f32 = np.float32
    seq_ids = np.asarray(seq_ids)
    out = np.empty((len(seq_ids), L, H2), f32)
    cores = np.where(seq_ids < SQ, seq_ids // QC, (seq_ids - SQ) // RC)
    pos = np.where(seq_ids < SQ, seq_ids % QC, QC + (seq_ids - SQ) % RC)
    for core in np.unique(cores):
        m = np.where(cores == core)[0]
        p = pos[m]
        f = hTf[core][:, :, :, p]        # (L, 2, 128, npos)
        b = hTb[core][:, :, :, p]
        out[m, :, 0:HID] = f.reshape(L, HID, len(p)).transpose(2, 0, 1)
        out[m, :, HID:H2] = b.reshape(L, HID, len(p)).transpose(2, 0, 1)
    return out


def kernel(**inputs):
    f32 = np.float32
    emb = np.asarray(inputs['emb'], f32)
    que_batch = np.asarray(inputs['que_batch'])
    que_asp = np.asarray(inputs['que_asp'])
    rev_batch = np.asarray(inputs['rev_batch'])
    rev_asp = np.asarray(inputs['rev_asp'])
    rev_opi = np.asarray(inputs['rev_opi'])
    que_mask = np.asarray(inputs['que_mask'], f32)
    que_asp_mask = np.asarray(inputs['que_asp_mask'], f32)
    rev_mask = np.asarray(inputs['rev_mask'], f32)
    rev_asp_mask = np.asarray(inputs['rev_asp_mask'])
    ratings = np.asarray(inputs['ratings'])
    rev_extend_vocab = np.asarray(inputs['rev_extend_vocab'])
    Wbil = np.asarray(inputs['Wbil'], f32)

    b_r = B * K

    q_a = emb[que_asp]                              # (B, NQ, D)
    r_a = emb[rev_asp]                              # (B, K, NR, D)
    r_o = emb[rev_opi]                              # (B, K, NR, D)
    e_q = emb[que_batch]                            # (B, LQ, D)
    e_r = emb[rev_batch.reshape(b_r, LR)]           # (b_r, LR, D)

    e_seq = np.empty((S, LQ, D), f32)
    asp = np.empty((S, D), f32)
    e_seq[:SQ] = np.repeat(e_q, NQ, axis=0)
    asp[:SQ] = q_a.reshape(SQ, D)
    e_seq[SQ:] = np.repeat(e_r, NR, axis=0)
    asp[SQ:] = r_a.reshape(SR, D)

    params = tuple(np.asarray(inputs[k], f32) for k in
                   ('Wih_f', 'Whh_f', 'bih_f', 'bhh_f',
                    'Wih_b', 'Whh_b', 'bih_b', 'bhh_b'))

    trace = bool(os.environ.get("BASS_LSTM_TRACE"))
    hTf, hTb, hNT, cN = _lstm_trainium(e_seq, asp, params, trace=trace)

    # ---- masked sums over t, from transposed layout (cheap) ----
    ones_mask = bool(np.all(que_mask == 1.0) and np.all(rev_mask == 1.0))
    if not ones_mask:
        mask_seq = np.empty((S, LQ), f32)
        mask_seq[:SQ] = np.repeat(que_mask, NQ, axis=0)
        mask_seq[SQ:] = np.repeat(rev_mask.reshape(b_r, LR), NR, axis=0)
    s_seq = np.empty((S, H2), f32)      # sum_t mask[t] * h[s, t, :]
    for core in range(NCORES):
        ids = _core_seq_ids(core)
        if ones_mask:
            sf = hTf[core].sum(0)       # (2, 128, SC)
            sb = hTb[core].sum(0)
        else:
            mk = mask_seq[ids]          # (SC, L)
            sf = np.einsum('tjps,st->jps', hTf[core], mk, optimize=True)
            sb = np.einsum('tjps,st->jps', hTb[core], mk, optimize=True)
        s_seq[ids, 0:HID] = sf.reshape(HID, SC).T
        s_seq[ids, HID:H2] = sb.reshape(HID, SC).T

    # ---- final states for question sequences ----
    hf = np.empty((SQ, HID), f32)
    hb = np.empty((SQ, HID), f32)
    cf = np.empty((SQ, HID), f32)
    cb = np.empty((SQ, HID), f32)
    for core in range(NCORES):
        qids = slice(QC * core, QC * (core + 1))
        hn = hNT[core]                  # (2, 2, 128, SC)
        hf[qids] = hn[0, :, :, 0:QC].reshape(HID, QC).T
        hb[qids] = hn[1, :, :, 0:QC].reshape(HID, QC).T
        cn = cN[core]                   # (2, SC, HID)
        cf[qids] = cn[0, 0:QC]
        cb[qids] = cn[1, 0:QC]

    qm = que_asp_mask
    s_st = np.stack([hf, hb]).reshape(2, B, NQ, HID)
    c_st = np.stack([cf, cb]).reshape(2, B, NQ, HID)
    s_out = (qm[None, :, :, None] * s_st).sum(2) / qm.sum(1)[None, :, None]
    c_out = (qm[None, :, :, None] * c_st).sum(2) / qm.sum(1)[None, :, None]

    # ---- question h (full, needed as output) ----
    h_q_flat = _gather_h(hTf, hTb, np.arange(SQ))            # (32, L, H2)
    h_q = qm[:, :, None, None] * h_q_flat.reshape(B, NQ, LQ, H2)

    # ---- attention sums / means ----
    s_q = qm[:, :, None] * s_seq[:SQ].reshape(B, NQ, H2)
    _s_q = s_q.sum(1) / qm.sum(1, keepdims=True)
    _q_a = q_a.sum(1) / qm.sum(1, keepdims=True)

    ram = np.asarray(rev_asp_mask, f32).reshape(b_r, NR)
    s_r = ram[:, :, None] * s_seq[SQ:].reshape(b_r, NR, H2)
    _s_r = s_r.sum(1) / ram.sum(1, keepdims=True)
    _r_a = r_a.reshape(b_r, NR, D).sum(1) / ram.sum(1, keepdims=True)

    # ---- bilinear scores + topk ----
    a_s_q = np.concatenate([_s_q, _q_a], axis=1)
    a_s_r = np.concatenate([_s_r, _r_a], axis=1).reshape(B, K, -1)
    scores = np.tanh(np.einsum('bd,de,bke->bk', a_s_q, Wbil, a_s_r,
                               optimize=True)).astype(f32)
    idx = np.argsort(-scores, axis=1, kind='stable')[:, :TOPK].astype(np.int32)
    rb = np.arange(B)[:, None]
    topk_scores = scores[rb, idx]

    # ---- gather h for selected reviews only ----
    # review seq global id = SQ + ((b*K + k)*NR + nr)
    rid = (SQ + ((np.arange(B)[:, None, None] * K + idx[:, :, None]) * NR
                 + np.arange(NR)[None, None, :]))    # (B, TOPK, NR)
    h_r5_flat = _gather_h(hTf, hTb, rid.reshape(-1))  # (B*TOPK*NR, L, H2)
    h_r5 = h_r5_flat.reshape(B, TOPK, NR, LR, H2)
    ram5 = ram.reshape(B, K, NR)[rb, idx]            # (B, TOPK, NR)
    h_r5 = ram5[..., None, None] * h_r5

    sel = lambda x: x[rb, idx]
    return (h_q.astype(f32), (s_out.astype(f32), c_out.astype(f32)),
            q_a.astype(f32),
            h_r5.astype(f32),
            sel(s_r.reshape(B, K, NR, H2)).astype(f32),
            sel(r_a), sel(r_o), sel(np.asarray(rev_asp_mask)),
            sel(rev_batch), sel(rev_mask), sel(ratings),
            sel(rev_extend_vocab), idx, topk_scores)


# revision 20
# speedup vs baseline: 1.2346x; 1.2346x over previous
"""Trainium2 Bass kernel for nn_AOEncoder (topk_masking).

Strategy (data-parallel over the 544 unified BiLSTM sequences):
  - The model's compute is dominated by two BiLSTMs sharing weights:
    question (B*NQ = 32 seqs) + review (B*K*NR = 512 seqs), all length 64,
    input = [token_emb(128) | aspect_emb(128)], hidden 256, bidirectional.
  - One unified batch of 544 sequences, sharded 68 per NeuronCore
    (4 question + 64 review). Forward and backward chains are staggered so
    PE / ACT / DVE / GPSIMD overlap across the two recurrences.
  - Per step+dir: gates(68,1024) = [e_t; a; hT] x W via K-tile f32r matmuls
    (1 cyc/row), sigmoid/tanh on ACT, c update on DVE+GPSIMD. The next-step
    stationary hT is produced in transposed space: PE transposes c and the
    o-gate, tanh(cT) runs on ACT from PSUM, and h^T = tanh(cT)*oT on DVE.
    h is written to DRAM transposed; the host only untransposes the
    sequences that appear in the output (question + top-k reviews).
  - The self-attention in the reference is degenerate (softmax over a
    size-1 axis => alpha = mask), so attention = masked sum over L; that,
    the bilinear scores, top-k and gathers are tiny and run on host numpy.
"""
import os
import sys
import numpy as np

for _p in ("/opt/trn_rl_repo", "/root/.axon_site/_ro/trn_rl_repo"):
    if _p not in sys.path and os.path.isdir(_p):
        sys.path.append(_p)

B, K, NQ, NR, LQ, LR = 8, 16, 4, 4, 64, 64
D, HID, V, TOPK = 128, 256, 50000, 5
H2 = 2 * HID
G4 = 4 * HID          # 1024 gate channels
SQ = B * NQ           # 32
SR = B * K * NR       # 512
S = SQ + SR           # 544
NCORES = 8
SC = S // NCORES      # 68 sequences per core
QC = SQ // NCORES     # 4 question seqs per core
RC = SR // NCORES     # 64 review seqs per core
L = LQ                # 64 steps

_cache = {}


def _build_nc(with_bias):
    import concourse.bass as bass
    import concourse.tile as tile
    from concourse import bacc, mybir

    f32 = mybir.dt.float32
    f32r = mybir.dt.float32r
    AF = mybir.ActivationFunctionType

    nc = bacc.Bacc(None, target_bir_lowering=False)

    eT_d = nc.dram_tensor("eT", [D, L, SC], f32r, kind="ExternalInput")
    aT_d = nc.dram_tensor("aT", [D, SC], f32r, kind="ExternalInput")
    # W[dir]: (128, 4, 1024): K-tiles [e, a, hT0, hT1] x gate cols [i,f,o,g]
    Wf_d = nc.dram_tensor("Wf", [128, 4, G4], f32r, kind="ExternalInput")
    Wb_d = nc.dram_tensor("Wb", [128, 4, G4], f32r, kind="ExternalInput")
    bias_d = nc.dram_tensor("bias", [1, 2, G4], f32r, kind="ExternalInput")
    ones_d = nc.dram_tensor("ones", [1, SC], f32r, kind="ExternalInput")
    ident_d = nc.dram_tensor("ident", [128, 128], f32r, kind="ExternalInput")

    # h output, transposed: hT[t, j, p, s] = h[s, t, 128*j + p] (per dir)
    hf_d = nc.dram_tensor("hTf", [L, 2, 128, SC], f32r, kind="ExternalOutput")
    hb_d = nc.dram_tensor("hTb", [L, 2, 128, SC], f32r, kind="ExternalOutput")
    # final states: hN transposed (dir, j, p, s); cN normal (dir, s, hid)
    hn_d = nc.dram_tensor("hNT", [2, 2, 128, SC], f32r, kind="ExternalOutput")
    cn_d = nc.dram_tensor("cN", [2, SC, HID], f32r, kind="ExternalOutput")

    with tile.TileContext(nc) as tc:
        with (
            tc.tile_pool(name="const", bufs=1) as constp,
            tc.tile_pool(name="state", bufs=2) as statep,
            tc.tile_pool(name="work", bufs=3) as workp,
            tc.tile_pool(name="psg0", bufs=2, space="PSUM") as psg0,
            tc.tile_pool(name="psg1", bufs=3, space="PSUM") as psg1,
            tc.tile_pool(name="pst", bufs=3, space="PSUM") as pst,
        ):
            eT = constp.tile([D, L, SC], f32r, tag="eT")
            aT = constp.tile([D, SC], f32r, tag="aT")
            W = {
                'f': constp.tile([128, 4, G4], f32r, tag="Wf", name="W_f"),
                'b': constp.tile([128, 4, G4], f32r, tag="Wb", name="W_b"),
            }
            bias_t = constp.tile([1, 2, G4], f32r, tag="bias")
            ones_t = constp.tile([1, SC], f32r, tag="ones")
            ident = constp.tile([128, 128], f32r, tag="ident")
            # first-needed-first: e/a weight K-tiles + boundary eT slices,
            # then identity/bias, then h weight K-tiles, then bulk eT
            nc.sync.dma_start(eT[:, 0, :], eT_d[:, 0, :])
            nc.scalar.dma_start(aT[:], aT_d[:])
            nc.sync.dma_start(W['f'][:, 0, 0:512], Wf_d[:, 0, 0:512])
            nc.scalar.dma_start(W['f'][:, 1, 0:512], Wf_d[:, 1, 0:512])
            nc.gpsimd.dma_start(W['f'][:, 0:2, 512:], Wf_d[:, 0:2, 512:])
            nc.gpsimd.dma_start(eT[:, L - 1, :], eT_d[:, L - 1, :])
            nc.scalar.dma_start(W['b'][:, 0:2, 0:512], Wb_d[:, 0:2, 0:512])
            nc.sync.dma_start(W['b'][:, 0:2, 512:], Wb_d[:, 0:2, 512:])
            nc.sync.dma_start(bias_t[:], bias_d[:])
            nc.sync.dma_start(ones_t[:], ones_d[:])
            nc.sync.dma_start(ident[:], ident_d[:])
            nc.sync.dma_start(W['f'][:, 2:4, :], Wf_d[:, 2:4, :])
            nc.sync.dma_start(W['b'][:, 2:4, :], Wb_d[:, 2:4, :])
            nc.sync.dma_start(eT[:, 1:8, :], eT_d[:, 1:8, :])
            nc.sync.dma_start(eT[:, 56:L - 1, :], eT_d[:, 56:L - 1, :])
            nc.sync.dma_start(eT[:, 8:32, :], eT_d[:, 8:32, :])
            nc.sync.dma_start(eT[:, 32:56, :], eT_d[:, 32:56, :])

            hT = {}
            c = {}
            g_ps = {}

            def mm(t, dr, di):
                te = t if dr == 'f' else (L - 1 - t)
                g0 = psg0.tile([SC, 512], f32, tag="g0", name="g0_" + dr)
                g1 = psg1.tile([SC, 512], f32, tag="g1", name="g1_" + dr)
                g_ps[dr] = (g0, g1)
                for nj, g in enumerate((g0, g1)):
                    nsl = slice(nj * 512, (nj + 1) * 512)
                    nc.tensor.matmul(g[:], eT[:, te, :], W[dr][:, 0, nsl],
                                     start=True, stop=False)
                    last = not (with_bias or t > 0)
                    nc.tensor.matmul(g[:], aT[:], W[dr][:, 1, nsl],
                                     start=False, stop=last)
                    if with_bias:
                        nc.tensor.matmul(g[:], ones_t[:],
                                         bias_t[:, di, nsl],
                                         start=False, stop=not t > 0)
                    if t > 0:
                        nc.tensor.matmul(g[:], hT[dr][:, 0, :],
                                         W[dr][:, 2, nsl],
                                         start=False, stop=False)
                        nc.tensor.matmul(g[:], hT[dr][:, 1, :],
                                         W[dr][:, 3, nsl],
                                         start=False, stop=True)

            def tail(t, dr, di):
                g0, g1 = g_ps[dr]
                # chunk0 = [f, g]; chunk1 = [i, o]
                sf = workp.tile([SC, HID], f32r, tag="sf", name="sf_" + dr)
                nc.scalar.activation(sf[:], g0[:, 0:HID], AF.Sigmoid)
                gg = workp.tile([SC, HID], f32r, tag="gg", name="gg_" + dr)
                nc.scalar.activation(gg[:], g0[:, HID:], AF.Tanh)
                sio = workp.tile([SC, 2 * HID], f32r, tag="sio",
                                 name="sio_" + dr)
                nc.scalar.activation(sio[:, 0:HID], g1[:, 0:HID], AF.Sigmoid)
                nc.scalar.activation(sio[:, HID:], g1[:, HID:], AF.Sigmoid)

                c_new = statep.tile([SC, HID], f32r, tag="c" + dr,
                                    name="c_" + dr)
                if t > 0:
                    fc = workp.tile([SC, HID], f32r, tag="fc",
                                    name="fc_" + dr)
                    nc.gpsimd.tensor_mul(fc[:], sf[:], c[dr][:])
                    ig = workp.tile([SC, HID], f32r, tag="ig",
                                    name="ig_" + dr)
                    nc.vector.tensor_mul(ig[:], sio[:, 0:HID], gg[:])
                    nc.vector.tensor_add(c_new[:], fc[:], ig[:])
                else:
                    nc.vector.tensor_mul(c_new[:], sio[:, 0:HID], gg[:])
                c[dr] = c_new

                # transposed tail: hT = tanh(cT) * oT  (oT, cT share a bank)
                ocT = pst.tile([128, 4, SC], f32r, tag="tps", name="ocT_" + dr)
                nc.tensor.transpose(ocT[:, 0, :], sio[:, HID:HID + 128],
                                    ident[0:SC, 0:SC])
                nc.tensor.transpose(ocT[:, 1, :], sio[:, HID + 128:],
                                    ident[0:SC, 0:SC])
                nc.tensor.transpose(ocT[:, 2, :], c_new[:, 0:128],
                                    ident[0:SC, 0:SC])
                nc.tensor.transpose(ocT[:, 3, :], c_new[:, 128:],
                                    ident[0:SC, 0:SC])
                tcT = workp.tile([128, 2, SC], f32r, tag="tcT",
                                 name="tcT_" + dr)
                nc.scalar.activation(tcT[:], ocT[:, 2:4, :], AF.Tanh)
                hT_new = statep.tile([128, 2, SC], f32r, tag="hT" + dr,
                                     name="hT_" + dr)
                nc.vector.tensor_mul(hT_new[:], tcT[:], ocT[:, 0:2, :])
                hT[dr] = hT_new

                # write h (transposed) to DRAM; bwd un-reversed
                h_d = hf_d if dr == 'f' else hb_d
                to = t if dr == 'f' else (L - 1 - t)
                nc.sync.dma_start(
                    h_d[to].rearrange("j p s -> p j s"), hT_new[:])
                if t == L - 1:
                    nc.sync.dma_start(
                        hn_d[di].rearrange("j p s -> p j s"), hT_new[:])
                    nc.sync.dma_start(cn_d[di], c_new[:])

            # software-pipelined emission: fwd and bwd staggered a half step
            mm(0, 'f', 0)
            mm(0, 'b', 1)
            tail(0, 'f', 0)
            for t in range(1, L):
                mm(t, 'f', 0)
                tail(t - 1, 'b', 1)
                mm(t, 'b', 1)
                tail(t, 'f', 0)
            tail(L - 1, 'b', 1)

    nc.compile()
    return nc


def _core_seq_ids(core):
    qs = np.arange(QC * core, QC * (core + 1))
    rs = SQ + np.arange(RC * core, RC * (core + 1))
    return np.concatenate([qs, rs])


def _lstm_trainium(e_seq, asp, params, trace=False):
    """e_seq: (S, L, D) f32, asp: (S, D) f32.
    Returns per-core transposed results:
      hTf, hTb: lists of (L, 2, 128, SC)  [t, j, p, s]
      hNT: list of (2, 2, 128, SC), cN: list of (2, SC, HID)."""
    from concourse.bass_utils import run_bass_kernel_spmd

    (Wih_f, Whh_f, bih_f, bhh_f, Wih_b, Whh_b, bih_b, bhh_b) = params
    f32 = np.float32
    bias_f = (bih_f + bhh_f).astype(f32)
    bias_b = (bih_b + bhh_b).astype(f32)
    with_bias = bool(np.any(bias_f) or np.any(bias_b))

    # gate permutation [f, g, i, o] from pytorch [i, f, g, o]
    perm = np.concatenate([np.arange(HID, 3 * HID),
                           np.arange(0, HID),
                           np.arange(3 * HID, 4 * HID)])

    def pack_W(Wih, Whh):
        # (128, 4, 1024): K-tiles [e, a, hT0, hT1], cols gate-permuted
        Wp_ih = Wih[perm]           # (1024, 256)
        Wp_hh = Whh[perm]           # (1024, 256)
        out = np.empty((128, 4, G4), f32)
        out[:, 0] = Wp_ih[:, 0:D].T
        out[:, 1] = Wp_ih[:, D:2 * D].T
        out[:, 2] = Wp_hh[:, 0:128].T
        out[:, 3] = Wp_hh[:, 128:256].T
        return out

    Wf = pack_W(Wih_f, Whh_f)
    Wb = pack_W(Wih_b, Whh_b)
    bias = np.stack([bias_f[perm], bias_b[perm]])[None]  # (1, 2, 1024)
    ident = np.eye(128, dtype=f32)
    ones = np.ones((1, SC), f32)

    key = with_bias
    if key not in _cache:
        _cache[key] = _build_nc(with_bias)
    nc = _cache[key]

    in_maps = []
    for core in range(NCORES):
        ids = _core_seq_ids(core)
        eT = np.ascontiguousarray(e_seq[ids].transpose(2, 1, 0))  # (D, L, SC)
        aT = np.ascontiguousarray(asp[ids].T)                     # (D, SC)
        in_maps.append({
            "eT": eT, "aT": aT, "Wf": Wf, "Wb": Wb,
            "bias": bias, "ones": ones, "ident": ident,
        })

    res = run_bass_kernel_spmd(nc, in_maps, core_ids=list(range(NCORES)),
                               trace=trace)
    if trace:
        _lstm_trainium.last_exec_ns = res.exec_time_ns

    hTf = [res.results[i]["hTf"] for i in range(NCORES)]
    hTb = [res.results[i]["hTb"] for i in range(NCORES)]
    hNT = [res.results[i]["hNT"] for i in range(NCORES)]
    cN = [res.results[i]["cN"] for i in range(NCORES)]
    return hTf, hTb, hNT, cN


_lstm_trainium.last_exec_ns = None


def _gather_h(hTf, hTb, seq_ids):
    """Assemble h (n, L, H2) for the given global sequence ids from the
    per-core transposed outputs."""
    f32 = np.float32
    seq_ids = np.asarray(seq_ids)
    out = np.empty((len(seq_ids), L, H2), f32)
    cores = np.where(seq_ids < SQ, seq_ids // QC, (seq_ids - SQ) // RC)
    pos = np.where(seq_ids < SQ, seq_ids % QC, QC + (seq_ids - SQ) % RC)
    for core in np.unique(cores):
        m = np.where(cores == core)[0]
        p = pos[m]
        f = hTf[core][:, :, :, p]        # (L, 2, 128, npos)
        b = hTb[core][:, :, :, p]
        out[m, :, 0:HID] = f.reshape(L, HID, len(p)).transpose(2, 0, 1)
        out[m, :, HID:H2] = b.reshape(L, HID, len(p)).transpose(2, 0, 1)
    return out


def kernel(**inputs):
    f32 = np.float32
    emb = np.asarray(inputs['emb'], f32)
    que_batch = np.asarray(inputs['que_batch'])
    que_asp = np.asarray(inputs['que_asp'])
    rev_batch = np.asarray(inputs['rev_batch'])
    rev_asp = np.asarray(inputs['rev_asp'])
    rev_opi = np.asarray(inputs['rev_opi'])
    que_mask = np.asarray(inputs['que_mask'], f32)
    que_asp_mask = np.asarray(inputs['que_asp_mask'], f32)
    rev_mask = np.asarray(inputs['rev_mask'], f32)
    rev_asp_mask = np.asarray(inputs['rev_asp_mask'])
    ratings = np.asarray(inputs['ratings'])
    rev_extend_vocab = np.asarray(inputs['rev_extend_vocab'])
    Wbil = np.asarray(inputs['Wbil'], f32)

    b_r = B * K

    q_a = emb[que_asp]                              # (B, NQ, D)
    r_a = emb[rev_asp]                              # (B, K, NR, D)
    r_o = emb[rev_opi]                              # (B, K, NR, D)
    e_q = emb[que_batch]                            # (B, LQ, D)
    e_r = emb[rev_batch.reshape(b_r, LR)]           # (b_r, LR, D)

    e_seq = np.empty((S, LQ, D), f32)
    asp = np.empty((S, D), f32)
    e_seq[:SQ] = np.repeat(e_q, NQ, axis=0)
    asp[:SQ] = q_a.reshape(SQ, D)
    e_seq[SQ:] = np.repeat(e_r, NR, axis=0)
    asp[SQ:] = r_a.reshape(SR, D)

    params = tuple(np.asarray(inputs[k], f32) for k in
                   ('Wih_f', 'Whh_f', 'bih_f', 'bhh_f',
                    'Wih_b', 'Whh_b', 'bih_b', 'bhh_b'))

    trace = bool(os.environ.get("BASS_LSTM_TRACE"))
    hTf, hTb, hNT, cN = _lstm_trainium(e_seq, asp, params, trace=trace)

    # ---- masked sums over t, from transposed layout (cheap) ----
    ones_mask = bool(np.all(que_mask == 1.0) and np.all(rev_mask == 1.0))
    if not ones_mask:
        mask_seq = np.empty((S, LQ), f32)
        mask_seq[:SQ] = np.repeat(que_mask, NQ, axis=0)
        mask_seq[SQ:] = np.repeat(rev_mask.reshape(b_r, LR), NR, axis=0)
    s_seq = np.empty((S, H2), f32)      # sum_t mask[t] * h[s, t, :]
    for core in range(NCORES):
        ids = _core_seq_ids(core)
        if ones_mask:
            sf = hTf[core].sum(0)       # (2, 128, SC)
            sb = hTb[core].sum(0)
        else:
            mk = mask_seq[ids]          # (SC, L)
            sf = np.einsum('tjps,st->jps', hTf[core], mk, optimize=True)
            sb = np.einsum('tjps,st->jps', hTb[core], mk, optimize=True)
        s_seq[ids, 0:HID] = sf.reshape(HID, SC).T
        s_seq[ids, HID:H2] = sb.reshape(HID, SC).T

    # ---- final states for question sequences ----
    hf = np.empty((SQ, HID), f32)
    hb = np.empty((SQ, HID), f32)
    cf = np.empty((SQ, HID), f32)
    cb = np.empty((SQ, HID), f32)
    for core in range(NCORES):
        qids = slice(QC * core, QC * (core + 1))
        hn = hNT[core]                  # (2, 2, 128, SC)
        hf[qids] = hn[0, :, :, 0:QC].reshape(HID, QC).T
        hb[qids] = hn[1, :, :, 0:QC].reshape(HID, QC).T
        cn = cN[core]                   # (2, SC, HID)
        cf[qids] = cn[0, 0:QC]
        cb[qids] = cn[1, 0:QC]

    qm = que_asp_mask
    s_st = np.stack([hf, hb]).reshape(2, B, NQ, HID)
    c_st = np.stack([cf, cb]).reshape(2, B, NQ, HID)
    s_out = (qm[None, :, :, None] * s_st).sum(2) / qm.sum(1)[None, :, None]
    c_out = (qm[None, :, :, None] * c_st).sum(2) / qm.sum(1)[None, :, None]

    # ---- question h (full, needed as output) ----
    h_q_flat = _gather_h(hTf, hTb, np.arange(SQ))            # (32, L, H2)
    h_q = qm[:, :, None, None] * h_q_flat.reshape(B, NQ, LQ, H2)

    # ---- attention sums / means ----
    s_q = qm[:, :, None] * s_seq[:SQ].reshape(B, NQ, H2)
    _s_q = s_q.sum(1) / qm.sum(1, keepdims=True)
    _q_a = q_a.sum(1) / qm.sum(1, keepdims=True)

    ram = np.asarray(rev_asp_mask, f32).reshape(b_r, NR)
    s_r = ram[:, :, None] * s_seq[SQ:].reshape(b_r, NR, H2)
    _s_r = s_r.sum(1) / ram.sum(1, keepdims=True)
    _r_a = r_a.reshape(b_r, NR, D).sum(1) / ram.sum(1, keepdims=True)

    # ---- bilinear scores + topk ----
    a_s_q = np.concatenate([_s_q, _q_a], axis=1)
    a_s_r = np.concatenate([_s_r, _r_a], axis=1).reshape(B, K, -1)
    scores = np.tanh(np.einsum('bd,de,bke->bk', a_s_q, Wbil, a_s_r,
                               optimize=True)).astype(f32)
    idx = np.argsort(-scores, axis=1, kind='stable')[:, :TOPK].astype(np.int32)
    rb = np.arange(B)[:, None]
    topk_scores = scores[rb, idx]

    # ---- gather h for selected reviews only ----
    # review seq global id = SQ + ((b*K + k)*NR + nr)
    rid = (SQ + ((np.arange(B)[:, None, None] * K + idx[:, :, None]) * NR
                 + np.arange(NR)[None, None, :]))    # (B, TOPK, NR)
    h_r5_flat = _gather_h(hTf, hTb, rid.reshape(-1))  # (B*TOPK*NR, L, H2)
    h_r5 = h_r5_flat.reshape(B, TOPK, NR, LR, H2)
    ram5 = ram.reshape(B, K, NR)[rb, idx]            # (B, TOPK, NR)
    h_r5 = ram5[..., None, None] * h_r5

    sel = lambda x: x[rb, idx]
    return (h_q.astype(f32), (s_out.astype(f32), c_out.astype(f32)),
            q_a.astype(f32),
            h_r5.astype(f32),
            sel(s_r.reshape(B, K, NR, H2)).astype(f32),
            sel(r_a), sel(r_o), sel(np.asarray(rev_asp_mask)),
            sel(rev_batch), sel(rev_mask), sel(ratings),
            sel(rev_extend_vocab), idx, topk_scores)


# revision 21
# speedup vs baseline: 1.2393x; 1.0038x over previous
"""Trainium2 Bass kernel for nn_AOEncoder (topk_masking).

Strategy (data-parallel over the 544 unified BiLSTM sequences):
  - The model's compute is dominated by two BiLSTMs sharing weights:
    question (B*NQ = 32 seqs) + review (B*K*NR = 512 seqs), all length 64,
    input = [token_emb(128) | aspect_emb(128)], hidden 256, bidirectional.
  - One unified batch of 544 sequences, sharded 68 per NeuronCore
    (4 question + 64 review). Forward and backward chains are staggered so
    PE / ACT / DVE / GPSIMD overlap across the two recurrences.
  - Per step+dir: gates(68,1024) = [e_t; a; hT] x W via K-tile f32r matmuls
    (1 cyc/row), sigmoid/tanh on ACT, c update on DVE+GPSIMD. The next-step
    stationary hT is produced in transposed space: PE transposes c and the
    o-gate, tanh(cT) runs on ACT from PSUM, and h^T = tanh(cT)*oT on DVE.
    h is written to DRAM transposed; the host only untransposes the
    sequences that appear in the output (question + top-k reviews).
  - The self-attention in the reference is degenerate (softmax over a
    size-1 axis => alpha = mask), so attention = masked sum over L; that,
    the bilinear scores, top-k and gathers are tiny and run on host numpy.
"""
import os
import sys
import numpy as np

for _p in ("/opt/trn_rl_repo", "/root/.axon_site/_ro/trn_rl_repo"):
    if _p not in sys.path and os.path.isdir(_p):
        sys.path.append(_p)

B, K, NQ, NR, LQ, LR = 8, 16, 4, 4, 64, 64
D, HID, V, TOPK = 128, 256, 50000, 5
H2 = 2 * HID
G4 = 4 * HID          # 1024 gate channels
SQ = B * NQ           # 32
SR = B * K * NR       # 512
S = SQ + SR           # 544
NCORES = 8
SC = S // NCORES      # 68 sequences per core
QC = SQ // NCORES     # 4 question seqs per core
RC = SR // NCORES     # 64 review seqs per core
L = LQ                # 64 steps

_cache = {}


def _build_nc(with_bias):
    import concourse.bass as bass
    import concourse.tile as tile
    from concourse import bacc, mybir

    f32 = mybir.dt.float32
    f32r = mybir.dt.float32r
    AF = mybir.ActivationFunctionType

    nc = bacc.Bacc(None, target_bir_lowering=False)

    eT_d = nc.dram_tensor("eT", [D, L, SC], f32r, kind="ExternalInput")
    aT_d = nc.dram_tensor("aT", [D, SC], f32r, kind="ExternalInput")
    # W[dir]: (128, 4, 1024): K-tiles [e, a, hT0, hT1] x gate cols [i,f,o,g]
    Wf_d = nc.dram_tensor("Wf", [128, 4, G4], f32r, kind="ExternalInput")
    Wb_d = nc.dram_tensor("Wb", [128, 4, G4], f32r, kind="ExternalInput")
    bias_d = nc.dram_tensor("bias", [1, 2, G4], f32r, kind="ExternalInput")
    ones_d = nc.dram_tensor("ones", [1, SC], f32r, kind="ExternalInput")
    ident_d = nc.dram_tensor("ident", [128, 128], f32r, kind="ExternalInput")

    # h output, transposed: hT[t, j, p, s] = h[s, t, 128*j + p] (per dir)
    hf_d = nc.dram_tensor("hTf", [L, 2, 128, SC], f32r, kind="ExternalOutput")
    hb_d = nc.dram_tensor("hTb", [L, 2, 128, SC], f32r, kind="ExternalOutput")
    # final states: hN transposed (dir, j, p, s); cN normal (dir, s, hid)
    hn_d = nc.dram_tensor("hNT", [2, 2, 128, SC], f32r, kind="ExternalOutput")
    cn_d = nc.dram_tensor("cN", [2, SC, HID], f32r, kind="ExternalOutput")

    with tile.TileContext(nc) as tc:
        with (
            tc.tile_pool(name="const", bufs=1) as constp,
            tc.tile_pool(name="state", bufs=2) as statep,
            tc.tile_pool(name="work", bufs=3) as workp,
            tc.tile_pool(name="psg0", bufs=3, space="PSUM") as psg0,
            tc.tile_pool(name="psg1", bufs=3, space="PSUM") as psg1,
            tc.tile_pool(name="pst", bufs=2, space="PSUM") as pst,
        ):
            eT = constp.tile([D, L, SC], f32r, tag="eT")
            aT = constp.tile([D, SC], f32r, tag="aT")
            W = {
                'f': constp.tile([128, 4, G4], f32r, tag="Wf", name="W_f"),
                'b': constp.tile([128, 4, G4], f32r, tag="Wb", name="W_b"),
            }
            bias_t = constp.tile([1, 2, G4], f32r, tag="bias")
            ones_t = constp.tile([1, SC], f32r, tag="ones")
            ident = constp.tile([128, 128], f32r, tag="ident")
            # first-needed-first: e/a weight K-tiles + boundary eT slices,
            # then identity/bias, then h weight K-tiles, then bulk eT
            nc.sync.dma_start(eT[:, 0, :], eT_d[:, 0, :])
            nc.scalar.dma_start(aT[:], aT_d[:])
            nc.sync.dma_start(W['f'][:, 0, 0:512], Wf_d[:, 0, 0:512])
            nc.scalar.dma_start(W['f'][:, 1, 0:512], Wf_d[:, 1, 0:512])
            nc.gpsimd.dma_start(W['f'][:, 0:2, 512:], Wf_d[:, 0:2, 512:])
            nc.gpsimd.dma_start(eT[:, L - 1, :], eT_d[:, L - 1, :])
            nc.scalar.dma_start(W['b'][:, 0:2, 0:512], Wb_d[:, 0:2, 0:512])
            nc.sync.dma_start(W['b'][:, 0:2, 512:], Wb_d[:, 0:2, 512:])
            nc.sync.dma_start(bias_t[:], bias_d[:])
            nc.sync.dma_start(ones_t[:], ones_d[:])
            nc.sync.dma_start(ident[:], ident_d[:])
            nc.sync.dma_start(W['f'][:, 2:4, :], Wf_d[:, 2:4, :])
            nc.sync.dma_start(W['b'][:, 2:4, :], Wb_d[:, 2:4, :])
            nc.sync.dma_start(eT[:, 1:8, :], eT_d[:, 1:8, :])
            nc.sync.dma_start(eT[:, 56:L - 1, :], eT_d[:, 56:L - 1, :])
            nc.sync.dma_start(eT[:, 8:32, :], eT_d[:, 8:32, :])
            nc.sync.dma_start(eT[:, 32:56, :], eT_d[:, 32:56, :])

            hT = {}
            c = {}
            g_ps = {}

            def mm(t, dr, di):
                te = t if dr == 'f' else (L - 1 - t)
                g0 = psg0.tile([SC, 512], f32, tag="g0", name="g0_" + dr)
                g1 = psg1.tile([SC, 512], f32, tag="g1", name="g1_" + dr)
                g_ps[dr] = (g0, g1)
                for nj, g in enumerate((g0, g1)):
                    nsl = slice(nj * 512, (nj + 1) * 512)
                    nc.tensor.matmul(g[:], eT[:, te, :], W[dr][:, 0, nsl],
                                     start=True, stop=False)
                    last = not (with_bias or t > 0)
                    nc.tensor.matmul(g[:], aT[:], W[dr][:, 1, nsl],
                                     start=False, stop=last)
                    if with_bias:
                        nc.tensor.matmul(g[:], ones_t[:],
                                         bias_t[:, di, nsl],
                                         start=False, stop=not t > 0)
                    if t > 0:
                        nc.tensor.matmul(g[:], hT[dr][:, 0, :],
                                         W[dr][:, 2, nsl],
                                         start=False, stop=False)
                        nc.tensor.matmul(g[:], hT[dr][:, 1, :],
                                         W[dr][:, 3, nsl],
                                         start=False, stop=True)

            def tail(t, dr, di):
                g0, g1 = g_ps[dr]
                # chunk0 = [f, g]; chunk1 = [i, o]
                sf = workp.tile([SC, HID], f32r, tag="sf", name="sf_" + dr)
                nc.scalar.activation(sf[:], g0[:, 0:HID], AF.Sigmoid)
                gg = workp.tile([SC, HID], f32r, tag="gg", name="gg_" + dr)
                nc.scalar.activation(gg[:], g0[:, HID:], AF.Tanh)
                sio = workp.tile([SC, 2 * HID], f32r, tag="sio",
                                 name="sio_" + dr)
                nc.scalar.activation(sio[:, 0:HID], g1[:, 0:HID], AF.Sigmoid)
                nc.scalar.activation(sio[:, HID:], g1[:, HID:], AF.Sigmoid)

                c_new = statep.tile([SC, HID], f32r, tag="c" + dr,
                                    name="c_" + dr)
                if t > 0:
                    fc = workp.tile([SC, HID], f32r, tag="fc",
                                    name="fc_" + dr)
                    nc.gpsimd.tensor_mul(fc[:], sf[:], c[dr][:])
                    ig = workp.tile([SC, HID], f32r, tag="ig",
                                    name="ig_" + dr)
                    nc.vector.tensor_mul(ig[:], sio[:, 0:HID], gg[:])
                    nc.vector.tensor_add(c_new[:], fc[:], ig[:])
                else:
                    nc.vector.tensor_mul(c_new[:], sio[:, 0:HID], gg[:])
                c[dr] = c_new

                # transposed tail: hT = tanh(cT) * oT  (oT, cT share a bank)
                ocT = pst.tile([128, 4, SC], f32r, tag="tps", name="ocT_" + dr)
                nc.tensor.transpose(ocT[:, 0, :], sio[:, HID:HID + 128],
                                    ident[0:SC, 0:SC])
                nc.tensor.transpose(ocT[:, 1, :], sio[:, HID + 128:],
                                    ident[0:SC, 0:SC])
                nc.tensor.transpose(ocT[:, 2, :], c_new[:, 0:128],
                                    ident[0:SC, 0:SC])
                nc.tensor.transpose(ocT[:, 3, :], c_new[:, 128:],
                                    ident[0:SC, 0:SC])
                tcT = workp.tile([128, 2, SC], f32r, tag="tcT",
                                 name="tcT_" + dr)
                nc.scalar.activation(tcT[:], ocT[:, 2:4, :], AF.Tanh)
                hT_new = statep.tile([128, 2, SC], f32r, tag="hT" + dr,
                                     name="hT_" + dr)
                nc.vector.tensor_mul(hT_new[:], tcT[:], ocT[:, 0:2, :])
                hT[dr] = hT_new

                # write h (transposed) to DRAM; bwd un-reversed
                h_d = hf_d if dr == 'f' else hb_d
                to = t if dr == 'f' else (L - 1 - t)
                nc.sync.dma_start(
                    h_d[to].rearrange("j p s -> p j s"), hT_new[:])
                if t == L - 1:
                    nc.sync.dma_start(
                        hn_d[di].rearrange("j p s -> p j s"), hT_new[:])
                    nc.sync.dma_start(cn_d[di], c_new[:])

            # software-pipelined emission: fwd and bwd staggered a half step
            mm(0, 'f', 0)
            mm(0, 'b', 1)
            tail(0, 'f', 0)
            for t in range(1, L):
                mm(t, 'f', 0)
                tail(t - 1, 'b', 1)
                mm(t, 'b', 1)
                tail(t, 'f', 0)
            tail(L - 1, 'b', 1)

    nc.compile()
    return nc


def _core_seq_ids(core):
    qs = np.arange(QC * core, QC * (core + 1))
    rs = SQ + np.arange(RC * core, RC * (core + 1))
    return np.concatenate([qs, rs])


def _lstm_trainium(e_seq, asp, params, trace=False):
    """e_seq: (S, L, D) f32, asp: (S, D) f32.
    Returns per-core transposed results:
      hTf, hTb: lists of (L, 2, 128, SC)  [t, j, p, s]
      hNT: list of (2, 2, 128, SC), cN: list of (2, SC, HID)."""
    from concourse.bass_utils import run_bass_kernel_spmd

    (Wih_f, Whh_f, bih_f, bhh_f, Wih_b, Whh_b, bih_b, bhh_b) = params
    f32 = np.float32
    bias_f = (bih_f + bhh_f).astype(f32)
    bias_b = (bih_b + bhh_b).astype(f32)
    with_bias = bool(np.any(bias_f) or np.any(bias_b))

    # gate permutation [f, g, i, o] from pytorch [i, f, g, o]
    perm = np.concatenate([np.arange(HID, 3 * HID),
                           np.arange(0, HID),
                           np.arange(3 * HID, 4 * HID)])

    def pack_W(Wih, Whh):
        # (128, 4, 1024): K-tiles [e, a, hT0, hT1], cols gate-permuted
        Wp_ih = Wih[perm]           # (1024, 256)
        Wp_hh = Whh[perm]           # (1024, 256)
        out = np.empty((128, 4, G4), f32)
        out[:, 0] = Wp_ih[:, 0:D].T
        out[:, 1] = Wp_ih[:, D:2 * D].T
        out[:, 2] = Wp_hh[:, 0:128].T
        out[:, 3] = Wp_hh[:, 128:256].T
        return out

    Wf = pack_W(Wih_f, Whh_f)
    Wb = pack_W(Wih_b, Whh_b)
    bias = np.stack([bias_f[perm], bias_b[perm]])[None]  # (1, 2, 1024)
    ident = np.eye(128, dtype=f32)
    ones = np.ones((1, SC), f32)

    key = with_bias
    if key not in _cache:
        _cache[key] = _build_nc(with_bias)
    nc = _cache[key]

    in_maps = []
    for core in range(NCORES):
        ids = _core_seq_ids(core)
        eT = np.ascontiguousarray(e_seq[ids].transpose(2, 1, 0))  # (D, L, SC)
        aT = np.ascontiguousarray(asp[ids].T)                     # (D, SC)
        in_maps.append({
            "eT": eT, "aT": aT, "Wf": Wf, "Wb": Wb,
            "bias": bias, "ones": ones, "ident": ident,
        })

    res = run_bass_kernel_spmd(nc, in_maps, core_ids=list(range(NCORES)),
                               trace=trace)
    if trace:
        _lstm_trainium.last_exec_ns = res.exec_time_ns

    hTf = [res.results[i]["hTf"] for i in range(NCORES)]
    hTb = [res.results[i]["hTb"] for i in range(NCORES)]
    hNT = [res.results[i]["hNT"] for i in range(NCORES)]
    cN = [res.results[i]["cN"] for i in range(NCORES)]
    return hTf, hTb, hNT, cN


_lstm_trainium.last_exec_ns = None


def _gather_h(hTf, hTb, seq_ids):
    """Assemble h (n, L, H2) for the given global sequence ids from the
    per-core transposed outputs."""
    f32 = np.float32
    seq_ids = np.asarray(seq_ids)
    out = np.empty((len(seq_ids), L, H2), f32)
    cores = np.where(seq_ids < SQ, seq_ids // QC, (seq_ids - SQ) // RC)
    pos = np.where(seq_ids < SQ, seq_ids % QC, QC + (seq_ids - SQ) % RC)
    for core in np.unique(cores):
        m = np.where(cores == core)[0]
        p = pos[m]
        f = hTf[core][:, :, :, p]        # (L, 2, 128, npos)
        b = hTb[core][:, :, :, p]
        out[m, :, 0:HID] = f.reshape(L, HID, len(p)).transpose(2, 0, 1)
        out[m, :, HID:H2] = b.reshape(L, HID, len(p)).transpose(2, 0, 1)
    return out


def kernel(**inputs):
    f32 = np.float32
    emb = np.asarray(inputs['emb'], f32)
    que_batch = np.asarray(inputs['que_batch'])
    que_asp = np.asarray(inputs['que_asp'])
    rev_batch = np.asarray(inputs['rev_batch'])
    rev_asp = np.asarray(inputs['rev_asp'])
    rev_opi = np.asarray(inputs['rev_opi'])
    que_mask = np.asarray(inputs['que_mask'], f32)
    que_asp_mask = np.asarray(inputs['que_asp_mask'], f32)
    rev_mask = np.asarray(inputs['rev_mask'], f32)
    rev_asp_mask = np.asarray(inputs['rev_asp_mask'])
    ratings = np.asarray(inputs['ratings'])
    rev_extend_vocab = np.asarray(inputs['rev_extend_vocab'])
    Wbil = np.asarray(inputs['Wbil'], f32)

    b_r = B * K

    q_a = emb[que_asp]                              # (B, NQ, D)
    r_a = emb[rev_asp]                              # (B, K, NR, D)
    r_o = emb[rev_opi]                              # (B, K, NR, D)
    e_q = emb[que_batch]                            # (B, LQ, D)
    e_r = emb[rev_batch.reshape(b_r, LR)]           # (b_r, LR, D)

    e_seq = np.empty((S, LQ, D), f32)
    asp = np.empty((S, D), f32)
    e_seq[:SQ] = np.repeat(e_q, NQ, axis=0)
    asp[:SQ] = q_a.reshape(SQ, D)
    e_seq[SQ:] = np.repeat(e_r, NR, axis=0)
    asp[SQ:] = r_a.reshape(SR, D)

    params = tuple(np.asarray(inputs[k], f32) for k in
                   ('Wih_f', 'Whh_f', 'bih_f', 'bhh_f',
                    'Wih_b', 'Whh_b', 'bih_b', 'bhh_b'))

    trace = bool(os.environ.get("BASS_LSTM_TRACE"))
    hTf, hTb, hNT, cN = _lstm_trainium(e_seq, asp, params, trace=trace)

    # ---- masked sums over t, from transposed layout (cheap) ----
    ones_mask = bool(np.all(que_mask == 1.0) and np.all(rev_mask == 1.0))
    if not ones_mask:
        mask_seq = np.empty((S, LQ), f32)
        mask_seq[:SQ] = np.repeat(que_mask, NQ, axis=0)
        mask_seq[SQ:] = np.repeat(rev_mask.reshape(b_r, LR), NR, axis=0)
    s_seq = np.empty((S, H2), f32)      # sum_t mask[t] * h[s, t, :]
    for core in range(NCORES):
        ids = _core_seq_ids(core)
        if ones_mask:
            sf = hTf[core].sum(0)       # (2, 128, SC)
            sb = hTb[core].sum(0)
        else:
            mk = mask_seq[ids]          # (SC, L)
            sf = np.einsum('tjps,st->jps', hTf[core], mk, optimize=True)
            sb = np.einsum('tjps,st->jps', hTb[core], mk, optimize=True)
        s_seq[ids, 0:HID] = sf.reshape(HID, SC).T
        s_seq[ids, HID:H2] = sb.reshape(HID, SC).T

    # ---- final states for question sequences ----
    hf = np.empty((SQ, HID), f32)
    hb = np.empty((SQ, HID), f32)
    cf = np.empty((SQ, HID), f32)
    cb = np.empty((SQ, HID), f32)
    for core in range(NCORES):
        qids = slice(QC * core, QC * (core + 1))
        hn = hNT[core]                  # (2, 2, 128, SC)
        hf[qids] = hn[0, :, :, 0:QC].reshape(HID, QC).T
        hb[qids] = hn[1, :, :, 0:QC].reshape(HID, QC).T
        cn = cN[core]                   # (2, SC, HID)
        cf[qids] = cn[0, 0:QC]
        cb[qids] = cn[1, 0:QC]

    qm = que_asp_mask
    s_st = np.stack([hf, hb]).reshape(2, B, NQ, HID)
    c_st = np.stack([cf, cb]).reshape(2, B, NQ, HID)
    s_out = (qm[None, :, :, None] * s_st).sum(2) / qm.sum(1)[None, :, None]
    c_out = (qm[None, :, :, None] * c_st).sum(2) / qm.sum(1)[None, :, None]

    # ---- question h (full, needed as output) ----
    h_q_flat = _gather_h(hTf, hTb, np.arange(SQ))            # (32, L, H2)
    h_q = qm[:, :, None, None] * h_q_flat.reshape(B, NQ, LQ, H2)

    # ---- attention sums / means ----
    s_q = qm[:, :, None] * s_seq[:SQ].reshape(B, NQ, H2)
    _s_q = s_q.sum(1) / qm.sum(1, keepdims=True)
    _q_a = q_a.sum(1) / qm.sum(1, keepdims=True)

    ram = np.asarray(rev_asp_mask, f32).reshape(b_r, NR)
    s_r = ram[:, :, None] * s_seq[SQ:].reshape(b_r, NR, H2)
    _s_r = s_r.sum(1) / ram.sum(1, keepdims=True)
    _r_a = r_a.reshape(b_r, NR, D).sum(1) / ram.sum(1, keepdims=True)

    # ---- bilinear scores + topk ----
    a_s_q = np.concatenate([_s_q, _q_a], axis=1)
    a_s_r = np.concatenate([_s_r, _r_a], axis=1).reshape(B, K, -1)
    scores = np.tanh(np.einsum('bd,de,bke->bk', a_s_q, Wbil, a_s_r,
                               optimize=True)).astype(f32)
    idx = np.argsort(-scores, axis=1, kind='stable')[:, :TOPK].astype(np.int32)
    rb = np.arange(B)[:, None]
    topk_scores = scores[rb, idx]

    # ---- gather h for selected reviews only ----
    # review seq global id = SQ + ((b*K + k)*NR + nr)
    rid = (SQ + ((np.arange(B)[:, None, None] * K + idx[:, :, None]) * NR
                 + np.arange(NR)[None, None, :]))    # (B, TOPK, NR)
    h_r5_flat = _gather_h(hTf, hTb, rid.reshape(-1))  # (B*TOPK*NR, L, H2)
    h_r5 = h_r5_flat.reshape(B, TOPK, NR, LR, H2)
    ram5 = ram.reshape(B, K, NR)[rb, idx]            # (B, TOPK, NR)
    h_r5 = ram5[..., None, None] * h_r5

    sel = lambda x: x[rb, idx]
    return (h_q.astype(f32), (s_out.astype(f32), c_out.astype(f32)),
            q_a.astype(f32),
            h_r5.astype(f32),
            sel(s_r.reshape(B, K, NR, H2)).astype(f32),
            sel(r_a), sel(r_o), sel(np.asarray(rev_asp_mask)),
            sel(rev_batch), sel(rev_mask), sel(ratings),
            sel(rev_extend_vocab), idx, topk_scores)


# revision 22
# speedup vs baseline: 1.2422x; 1.0023x over previous
"""Trainium2 Bass kernel for nn_AOEncoder (topk_masking).

Strategy (data-parallel over the 544 unified BiLSTM sequences):
  - The model's compute is dominated by two BiLSTMs sharing weights:
    question (B*NQ = 32 seqs) + review (B*K*NR = 512 seqs), all length 64,
    input = [token_emb(128) | aspect_emb(128)], hidden 256, bidirectional.
  - One unified batch of 544 sequences, sharded 68 per NeuronCore
    (4 question + 64 review). Forward and backward chains are staggered so
    PE / ACT / DVE / GPSIMD overlap across the two recurrences.
  - Per step+dir: gates(68,1024) = [e_t; a; hT] x W via K-tile f32r matmuls
    (1 cyc/row), sigmoid/tanh on ACT, c update on DVE+GPSIMD. The next-step
    stationary hT is produced in transposed space: PE transposes c and the
    o-gate, tanh(cT) runs on ACT from PSUM, and h^T = tanh(cT)*oT on DVE.
    h is written to DRAM transposed; the host only untransposes the
    sequences that appear in the output (question + top-k reviews).
  - The self-attention in the reference is degenerate (softmax over a
    size-1 axis => alpha = mask), so attention = masked sum over L; that,
    the bilinear scores, top-k and gathers are tiny and run on host numpy.
"""
import os
import sys
import numpy as np

for _p in ("/opt/trn_rl_repo", "/root/.axon_site/_ro/trn_rl_repo"):
    if _p not in sys.path and os.path.isdir(_p):
        sys.path.append(_p)

B, K, NQ, NR, LQ, LR = 8, 16, 4, 4, 64, 64
D, HID, V, TOPK = 128, 256, 50000, 5
H2 = 2 * HID
G4 = 4 * HID          # 1024 gate channels
SQ = B * NQ           # 32
SR = B * K * NR       # 512
S = SQ + SR           # 544
NCORES = 8
SC = S // NCORES      # 68 sequences per core
QC = SQ // NCORES     # 4 question seqs per core
RC = SR // NCORES     # 64 review seqs per core
L = LQ                # 64 steps

_cache = {}


def _build_nc(with_bias):
    import concourse.bass as bass
    import concourse.tile as tile
    from concourse import bacc, mybir

    f32 = mybir.dt.float32
    f32r = mybir.dt.float32r
    AF = mybir.ActivationFunctionType

    nc = bacc.Bacc(None, target_bir_lowering=False)

    eT_d = nc.dram_tensor("eT", [D, L, SC], f32r, kind="ExternalInput")
    aT_d = nc.dram_tensor("aT", [D, SC], f32r, kind="ExternalInput")
    # W[dir]: (128, 4, 1024): K-tiles [e, a, hT0, hT1] x gate cols [i,f,o,g]
    Wf_d = nc.dram_tensor("Wf", [128, 4, G4], f32r, kind="ExternalInput")
    Wb_d = nc.dram_tensor("Wb", [128, 4, G4], f32r, kind="ExternalInput")
    bias_d = nc.dram_tensor("bias", [1, 2, G4], f32r, kind="ExternalInput")
    ones_d = nc.dram_tensor("ones", [1, SC], f32r, kind="ExternalInput")
    ident_d = nc.dram_tensor("ident", [128, 128], f32r, kind="ExternalInput")

    # h output, transposed: hT[t, j, p, s] = h[s, t, 128*j + p] (per dir)
    hf_d = nc.dram_tensor("hTf", [L, 2, 128, SC], f32r, kind="ExternalOutput")
    hb_d = nc.dram_tensor("hTb", [L, 2, 128, SC], f32r, kind="ExternalOutput")
    # final states: hN transposed (dir, j, p, s); cN normal (dir, s, hid)
    hn_d = nc.dram_tensor("hNT", [2, 2, 128, SC], f32r, kind="ExternalOutput")
    cn_d = nc.dram_tensor("cN", [2, SC, HID], f32r, kind="ExternalOutput")

    with tile.TileContext(nc) as tc:
        with (
            tc.tile_pool(name="const", bufs=1) as constp,
            tc.tile_pool(name="state", bufs=2) as statep,
            tc.tile_pool(name="work", bufs=3) as workp,
            tc.tile_pool(name="psg0", bufs=2, space="PSUM") as psg0,
            tc.tile_pool(name="psg1", bufs=3, space="PSUM") as psg1,
            tc.tile_pool(name="pst", bufs=3, space="PSUM") as pst,
        ):
            eT = constp.tile([D, L, SC], f32r, tag="eT")
            aT = constp.tile([D, SC], f32r, tag="aT")
            W = {
                'f': constp.tile([128, 4, G4], f32r, tag="Wf", name="W_f"),
                'b': constp.tile([128, 4, G4], f32r, tag="Wb", name="W_b"),
            }
            bias_t = constp.tile([1, 2, G4], f32r, tag="bias")
            ones_t = constp.tile([1, SC], f32r, tag="ones")
            ident = constp.tile([128, 128], f32r, tag="ident")
            # first-needed-first: e/a weight K-tiles + boundary eT slices,
            # then identity/bias, then h weight K-tiles, then bulk eT
            nc.sync.dma_start(eT[:, 0, :], eT_d[:, 0, :])
            nc.scalar.dma_start(aT[:], aT_d[:])
            nc.sync.dma_start(W['f'][:, 0, 0:512], Wf_d[:, 0, 0:512])
            nc.scalar.dma_start(W['f'][:, 1, 0:512], Wf_d[:, 1, 0:512])
            nc.gpsimd.dma_start(W['f'][:, 0:2, 512:], Wf_d[:, 0:2, 512:])
            nc.gpsimd.dma_start(eT[:, L - 1, :], eT_d[:, L - 1, :])
            nc.scalar.dma_start(W['b'][:, 0:2, 0:512], Wb_d[:, 0:2, 0:512])
            nc.sync.dma_start(W['b'][:, 0:2, 512:], Wb_d[:, 0:2, 512:])
            nc.sync.dma_start(bias_t[:], bias_d[:])
            nc.sync.dma_start(ones_t[:], ones_d[:])
            nc.sync.dma_start(ident[:], ident_d[:])
            nc.sync.dma_start(W['f'][:, 2:4, :], Wf_d[:, 2:4, :])
            nc.sync.dma_start(W['b'][:, 2:4, :], Wb_d[:, 2:4, :])
            nc.sync.dma_start(eT[:, 1:8, :], eT_d[:, 1:8, :])
            nc.sync.dma_start(eT[:, 56:L - 1, :], eT_d[:, 56:L - 1, :])
            nc.sync.dma_start(eT[:, 8:32, :], eT_d[:, 8:32, :])
            nc.sync.dma_start(eT[:, 32:56, :], eT_d[:, 32:56, :])

            hT = {}
            c = {}
            g_ps = {}

            def mm(t, dr, di):
                te = t if dr == 'f' else (L - 1 - t)
                g0 = psg0.tile([SC, 512], f32, tag="g0", name="g0_" + dr)
                g1 = psg1.tile([SC, 512], f32, tag="g1", name="g1_" + dr)
                g_ps[dr] = (g0, g1)
                for nj, g in enumerate((g0, g1)):
                    nsl = slice(nj * 512, (nj + 1) * 512)
                    nc.tensor.matmul(g[:], eT[:, te, :], W[dr][:, 0, nsl],
                                     start=True, stop=False)
                    last = not (with_bias or t > 0)
                    nc.tensor.matmul(g[:], aT[:], W[dr][:, 1, nsl],
                                     start=False, stop=last)
                    if with_bias:
                        nc.tensor.matmul(g[:], ones_t[:],
                                         bias_t[:, di, nsl],
                                         start=False, stop=not t > 0)
                    if t > 0:
                        nc.tensor.matmul(g[:], hT[dr][:, 0, :],
                                         W[dr][:, 2, nsl],
                                         start=False, stop=False)
                        nc.tensor.matmul(g[:], hT[dr][:, 1, :],
                                         W[dr][:, 3, nsl],
                                         start=False, stop=True)

            def tail(t, dr, di):
                g0, g1 = g_ps[dr]
                # chunk0 = [f, g]; chunk1 = [i, o]
                sf = workp.tile([SC, HID], f32r, tag="sf", name="sf_" + dr)
                nc.scalar.activation(sf[:], g0[:, 0:HID], AF.Sigmoid)
                gg = workp.tile([SC, HID], f32r, tag="gg", name="gg_" + dr)
                nc.scalar.activation(gg[:], g0[:, HID:], AF.Tanh)
                sio = workp.tile([SC, 2 * HID], f32r, tag="sio",
                                 name="sio_" + dr)
                nc.scalar.activation(sio[:, 0:HID], g1[:, 0:HID], AF.Sigmoid)
                nc.scalar.activation(sio[:, HID:], g1[:, HID:], AF.Sigmoid)

                c_new = statep.tile([SC, HID], f32r, tag="c" + dr,
                                    name="c_" + dr)
                if t > 0:
                    fc = workp.tile([SC, HID], f32r, tag="fc",
                                    name="fc_" + dr)
                    nc.gpsimd.tensor_mul(fc[:], sf[:], c[dr][:])
                    ig = workp.tile([SC, HID], f32r, tag="ig",
                                    name="ig_" + dr)
                    nc.vector.tensor_mul(ig[:], sio[:, 0:HID], gg[:])
                    nc.vector.tensor_add(c_new[:], fc[:], ig[:])
                else:
                    nc.vector.tensor_mul(c_new[:], sio[:, 0:HID], gg[:])
                c[dr] = c_new

                # transposed tail: hT = tanh(cT) * oT  (oT, cT share a bank)
                ocT = pst.tile([128, 4, SC], f32r, tag="tps", name="ocT_" + dr)
                nc.tensor.transpose(ocT[:, 0, :], sio[:, HID:HID + 128],
                                    ident[0:SC, 0:SC])
                nc.tensor.transpose(ocT[:, 1, :], sio[:, HID + 128:],
                                    ident[0:SC, 0:SC])
                nc.tensor.transpose(ocT[:, 2, :], c_new[:, 0:128],
                                    ident[0:SC, 0:SC])
                nc.tensor.transpose(ocT[:, 3, :], c_new[:, 128:],
                                    ident[0:SC, 0:SC])
                tcT = workp.tile([128, 2, SC], f32r, tag="tcT",
                                 name="tcT_" + dr)
                nc.scalar.activation(tcT[:], ocT[:, 2:4, :], AF.Tanh)
                hT_new = statep.tile([128, 2, SC], f32r, tag="hT" + dr,
                                     name="hT_" + dr)
                nc.vector.tensor_mul(hT_new[:], tcT[:], ocT[:, 0:2, :])
                hT[dr] = hT_new

                # write h (transposed) to DRAM; bwd un-reversed
                h_d = hf_d if dr == 'f' else hb_d
                to = t if dr == 'f' else (L - 1 - t)
                nc.sync.dma_start(
                    h_d[to].rearrange("j p s -> p j s"), hT_new[:])
                if t == L - 1:
                    nc.sync.dma_start(
                        hn_d[di].rearrange("j p s -> p j s"), hT_new[:])
                    nc.sync.dma_start(cn_d[di], c_new[:])

            # software-pipelined emission: fwd and bwd staggered a half step
            mm(0, 'f', 0)
            mm(0, 'b', 1)
            tail(0, 'f', 0)
            for t in range(1, L):
                mm(t, 'f', 0)
                tail(t - 1, 'b', 1)
                mm(t, 'b', 1)
                tail(t, 'f', 0)
            tail(L - 1, 'b', 1)

    nc.compile()
    return nc


def _core_seq_ids(core):
    qs = np.arange(QC * core, QC * (core + 1))
    rs = SQ + np.arange(RC * core, RC * (core + 1))
    return np.concatenate([qs, rs])


def _lstm_trainium(e_seq, asp, params, trace=False):
    """e_seq: (S, L, D) f32, asp: (S, D) f32.
    Returns per-core transposed results:
      hTf, hTb: lists of (L, 2, 128, SC)  [t, j, p, s]
      hNT: list of (2, 2, 128, SC), cN: list of (2, SC, HID)."""
    from concourse.bass_utils import run_bass_kernel_spmd

    (Wih_f, Whh_f, bih_f, bhh_f, Wih_b, Whh_b, bih_b, bhh_b) = params
    f32 = np.float32
    bias_f = (bih_f + bhh_f).astype(f32)
    bias_b = (bih_b + bhh_b).astype(f32)
    with_bias = bool(np.any(bias_f) or np.any(bias_b))

    # gate permutation [f, g, i, o] from pytorch [i, f, g, o]
    perm = np.concatenate([np.arange(HID, 3 * HID),
                           np.arange(0, HID),
                           np.arange(3 * HID, 4 * HID)])

    def pack_W(Wih, Whh):
        # (128, 4, 1024): K-tiles [e, a, hT0, hT1], cols gate-permuted
        Wp_ih = Wih[perm]           # (1024, 256)
        Wp_hh = Whh[perm]           # (1024, 256)
        out = np.empty((128, 4, G4), f32)
        out[:, 0] = Wp_ih[:, 0:D].T
        out[:, 1] = Wp_ih[:, D:2 * D].T
        out[:, 2] = Wp_hh[:, 0:128].T
        out[:, 3] = Wp_hh[:, 128:256].T
        return out

    Wf = pack_W(Wih_f, Whh_f)
    Wb = pack_W(Wih_b, Whh_b)
    bias = np.stack([bias_f[perm], bias_b[perm]])[None]  # (1, 2, 1024)
    ident = np.eye(128, dtype=f32)
    ones = np.ones((1, SC), f32)

    key = with_bias
    if key not in _cache:
        _cache[key] = _build_nc(with_bias)
    nc = _cache[key]

    in_maps = []
    for core in range(NCORES):
        ids = _core_seq_ids(core)
        eT = np.ascontiguousarray(e_seq[ids].transpose(2, 1, 0))  # (D, L, SC)
        aT = np.ascontiguousarray(asp[ids].T)                     # (D, SC)
        in_maps.append({
            "eT": eT, "aT": aT, "Wf": Wf, "Wb": Wb,
            "bias": bias, "ones": ones, "ident": ident,
        })

    res = run_bass_kernel_spmd(nc, in_maps, core_ids=list(range(NCORES)),
                               trace=trace)
    if trace:
        _lstm_trainium.last_exec_ns = res.exec_time_ns

    hTf = [res.results[i]["hTf"] for i in range(NCORES)]
    hTb = [res.results[i]["hTb"] for i in range(NCORES)]
    hNT = [res.results[i]["hNT"] for i in range(NCORES)]
    cN = [res.results[i]["cN"] for i in range(NCORES)]
    return hTf, hTb, hNT, cN


_lstm_trainium.last_exec_ns = None


def _gather_h(hTf, hTb, seq_ids):
    """Assemble h (n, L, H2) for the given global sequence ids from the
    per-core transposed outputs."""
    f32 = np.float32
    seq_ids = np.asarray(seq_ids)
    out = np.empty((len(seq_ids), L, H2), f32)
    cores = np.where(seq_ids < SQ, seq_ids // QC, (seq_ids - SQ) // RC)
    pos = np.where(seq_ids < SQ, seq_ids % QC, QC + (seq_ids - SQ) % RC)
    for core in np.unique(cores):
        m = np.where(cores == core)[0]
        p = pos[m]
        f = hTf[core][:, :, :, p]        # (L, 2, 128, npos)
        b = hTb[core][:, :, :, p]
        out[m, :, 0:HID] = f.reshape(L, HID, len(p)).transpose(2, 0, 1)
        out[m, :, HID:H2] = b.reshape(L, HID, len(p)).transpose(2, 0, 1)
    return out


def kernel(**inputs):
    f32 = np.float32
    emb = np.asarray(inputs['emb'], f32)
    que_batch = np.asarray(inputs['que_batch'])
    que_asp = np.asarray(inputs['que_asp'])
    rev_batch = np.asarray(inputs['rev_batch'])
    rev_asp = np.asarray(inputs['rev_asp'])
    rev_opi = np.asarray(inputs['rev_opi'])
    que_mask = np.asarray(inputs['que_mask'], f32)
    que_asp_mask = np.asarray(inputs['que_asp_mask'], f32)
    rev_mask = np.asarray(inputs['rev_mask'], f32)
    rev_asp_mask = np.asarray(inputs['rev_asp_mask'])
    ratings = np.asarray(inputs['ratings'])
    rev_extend_vocab = np.asarray(inputs['rev_extend_vocab'])
    Wbil = np.asarray(inputs['Wbil'], f32)

    b_r = B * K

    q_a = emb[que_asp]                              # (B, NQ, D)
    r_a = emb[rev_asp]                              # (B, K, NR, D)
    r_o = emb[rev_opi]                              # (B, K, NR, D)
    e_q = emb[que_batch]                            # (B, LQ, D)
    e_r = emb[rev_batch.reshape(b_r, LR)]           # (b_r, LR, D)

    e_seq = np.empty((S, LQ, D), f32)
    asp = np.empty((S, D), f32)
    e_seq[:SQ] = np.repeat(e_q, NQ, axis=0)
    asp[:SQ] = q_a.reshape(SQ, D)
    e_seq[SQ:] = np.repeat(e_r, NR, axis=0)
    asp[SQ:] = r_a.reshape(SR, D)

    params = tuple(np.asarray(inputs[k], f32) for k in
                   ('Wih_f', 'Whh_f', 'bih_f', 'bhh_f',
                    'Wih_b', 'Whh_b', 'bih_b', 'bhh_b'))

    trace = bool(os.environ.get("BASS_LSTM_TRACE"))
    hTf, hTb, hNT, cN = _lstm_trainium(e_seq, asp, params, trace=trace)

    # ---- masked sums over t, from transposed layout (cheap) ----
    ones_mask = bool(np.all(que_mask == 1.0) and np.all(rev_mask == 1.0))
    if not ones_mask:
        mask_seq = np.empty((S, LQ), f32)
        mask_seq[:SQ] = np.repeat(que_mask, NQ, axis=0)
        mask_seq[SQ:] = np.repeat(rev_mask.reshape(b_r, LR), NR, axis=0)
    s_seq = np.empty((S, H2), f32)      # sum_t mask[t] * h[s, t, :]
    for core in range(NCORES):
        ids = _core_seq_ids(core)
        if ones_mask:
            sf = hTf[core].sum(0)       # (2, 128, SC)
            sb = hTb[core].sum(0)
        else:
            mk = mask_seq[ids]          # (SC, L)
            sf = np.einsum('tjps,st->jps', hTf[core], mk, optimize=True)
            sb = np.einsum('tjps,st->jps', hTb[core], mk, optimize=True)
        s_seq[ids, 0:HID] = sf.reshape(HID, SC).T
        s_seq[ids, HID:H2] = sb.reshape(HID, SC).T

    # ---- final states for question sequences ----
    hf = np.empty((SQ, HID), f32)
    hb = np.empty((SQ, HID), f32)
    cf = np.empty((SQ, HID), f32)
    cb = np.empty((SQ, HID), f32)
    for core in range(NCORES):
        qids = slice(QC * core, QC * (core + 1))
        hn = hNT[core]                  # (2, 2, 128, SC)
        hf[qids] = hn[0, :, :, 0:QC].reshape(HID, QC).T
        hb[qids] = hn[1, :, :, 0:QC].reshape(HID, QC).T
        cn = cN[core]                   # (2, SC, HID)
        cf[qids] = cn[0, 0:QC]
        cb[qids] = cn[1, 0:QC]

    qm = que_asp_mask
    s_st = np.stack([hf, hb]).reshape(2, B, NQ, HID)
    c_st = np.stack([cf, cb]).reshape(2, B, NQ, HID)
    s_out = (qm[None, :, :, None] * s_st).sum(2) / qm.sum(1)[None, :, None]
    c_out = (qm[None, :, :, None] * c_st).sum(2) / qm.sum(1)[None, :, None]

    # ---- question h (full, needed as output) ----
    h_q_flat = _gather_h(hTf, hTb, np.arange(SQ))            # (32, L, H2)
    h_q = qm[:, :, None, None] * h_q_flat.reshape(B, NQ, LQ, H2)

    # ---- attention sums / means ----
    s_q = qm[:, :, None] * s_seq[:SQ].reshape(B, NQ, H2)
    _s_q = s_q.sum(1) / qm.sum(1, keepdims=True)
    _q_a = q_a.sum(1) / qm.sum(1, keepdims=True)

    ram = np.asarray(rev_asp_mask, f32).reshape(b_r, NR)
    s_r = ram[:, :, None] * s_seq[SQ:].reshape(b_r, NR, H2)
    _s_r = s_r.sum(1) / ram.sum(1, keepdims=True)
    _r_a = r_a.reshape(b_r, NR, D).sum(1) / ram.sum(1, keepdims=True)

    # ---- bilinear scores + topk ----
    a_s_q = np.concatenate([_s_q, _q_a], axis=1)
    a_s_r = np.concatenate([_s_r, _r_a], axis=1).reshape(B, K, -1)
    scores = np.tanh(np.einsum('bd,de,bke->bk', a_s_q, Wbil, a_s_r,
                               optimize=True)).astype(f32)
    idx = np.argsort(-scores, axis=1, kind='stable')[:, :TOPK].astype(np.int32)
    rb = np.arange(B)[:, None]
    topk_scores = scores[rb, idx]

    # ---- gather h for selected reviews only ----
    # review seq global id = SQ + ((b*K + k)*NR + nr)
    rid = (SQ + ((np.arange(B)[:, None, None] * K + idx[:, :, None]) * NR
                 + np.arange(NR)[None, None, :]))    # (B, TOPK, NR)
    h_r5_flat = _gather_h(hTf, hTb, rid.reshape(-1))  # (B*TOPK*NR, L, H2)
    h_r5 = h_r5_flat.reshape(B, TOPK, NR, LR, H2)
    ram5 = ram.reshape(B, K, NR)[rb, idx]            # (B, TOPK, NR)
    h_r5 = ram5[..., None, None] * h_r5

    sel = lambda x: x[rb, idx]
    return (h_q.astype(f32), (s_out.astype(f32), c_out.astype(f32)),
            q_a.astype(f32),
            h_r5.astype(f32),
            sel(s_r.reshape(B, K, NR, H2)).astype(f32),
            sel(r_a), sel(r_o), sel(np.asarray(rev_asp_mask)),
            sel(rev_batch), sel(rev_mask), sel(ratings),
            sel(rev_extend_vocab), idx, topk_scores)
